# revision 2
# baseline (speedup 1.0000x reference)
"""Trainium2 Bass kernel for nn_AttentionHead (B=8, S=4096, D=128).

Sharding: data-parallel over the batch dim — 1 batch element per NeuronCore,
8 cores, SPMD (same NEFF, different x slice), weights replicated. No
collectives.

Structure (v7). The kernel is ACT-bound: softmax needs S^2 = 16.7M exps per
core ~= 109us of Activation-engine time (1.2GHz x 128 lanes); everything
else is scheduled to hide under the exp stream and to minimize the prologue
before the first exp.

Attention (transposed scores, no exp transposes):
    scoresT[jt, q_group] = kT_tile^T @ qT_group  (groups of 512 queries)
so ACT's exp writes f16 straight into the [j, q] layout PV needs for lhsT.
Flat stream over n = g*16 + jp: QK pair (N=512) -> exp [128, 1024] from a
2-bank PSUM tile -> 8 PV matmuls for stream slot n-2. All four PV
accumulators of a group are live at once (pvps bufs=4 x 1 bank; qkps
bufs=2 x 2 banks), so PV chases exp inside its own group and the tail
after the last exp is one PV slot. PV's rhs is v16 with an appended ones
column: column P of each accumulator accumulates the softmax denominator.
Normalize by 1/rowsum on DVE; one batched output DMA per group (sync ring;
SWDGE would pay ~1us descriptor-gen per DMA on GPSIMD, and the scalar
ring's DMAs issue from the ACT sequencer, head-of-line blocking exps).

Prologue (everything before the first exp; target ~27us). Scheduling facts
(from the cost model / observed traces): an xbar DmaTransposeAnt waits for
ALL prior in-flight DMA completions (+~0.9us sem prop), each engine
executes its instructions in order, and per-queue DMAs serialize on the
DMA engines. Hence:
  - LN params + weights load first on the scalar ring (tiny; land <5us,
    before any transpose needs the DMA device); x cast-loads (f32->f16
    SWDGE) on the GPSIMD ring in 4 quarter chunks.
  - sync ring carries ONLY xbar transposes, in need-order: wt_k, the 4 x
    quarters (independent tiles so the k projections chase them), wt_q,
    wt_v, then post-LN kT halves / qT quarters.
  - pass A is k-first: project k for all 32 tiles (PE), stage raw k f16
    (ACT), bn stats from the staged f16 (DVE), one batched
    rsqrt = exp(-0.5*ln(var+eps)) (Ln/Exp share the ACT table set with the
    attention Exp -> no table thrash), LN apply via DVE tensor_scalar (two
    per-partition scalars), xbar-transpose halves, LN weight/bias fold on
    DVE (fast mode) right behind each transpose.
  - q tiles 0-7 get their own staging + rsqrt + apply + transpose + fold
    chain immediately after k's (attention group 0 only needs qT quarter
    0); v is projected/evacuated on DVE meanwhile. q quarters 1-3 are
    finished INSIDE the attention stream (their stats/rsqrt/apply/
    transpose/fold splice into early stream slots; group 2g only needs qT
    quarter g, due ~16.6*2g us after attention starts).

All SBUF pools stay open for the whole kernel (SBUF-slot reuse attaches
release waits to DMAs loading into recycled space; walrus rejects DMAs
with too many sync waits). Only PSUM pools are scoped. All xbar transposes
go on the sync HWDGE ring (concurrent transposes on the sync+scalar rings
corrupt data on HW).
"""

import math

import numpy as np

from concourse import bacc
import concourse.mybir as mybir
import concourse.tile as tile
from concourse.bass_utils import run_bass_kernel_spmd

F16 = mybir.dt.float16
F32 = mybir.dt.float32
AF = mybir.ActivationFunctionType
ALU = mybir.AluOpType

B, S, D = 8, 4096, 128
P = 128
NT = S // P    # 32 s-tiles
NQ = 4         # x/q quarter chunks
TPQ = NT // NQ  # 8 tiles per quarter
EPS = 1e-5
ISQRT_D = 1.0 / math.sqrt(D)
N_CORES = 8
GW = 512           # queries per attention group
NG = S // GW       # 8 groups
TPG = GW // P      # 4 i-tiles per group
JP = NT // 2       # 16 jt-pairs (= exp ops = stream slots) per group


def _build_attention(tc, out_d, x_d, w_d, ln_d):
    """Emit the single-core attention program.

    out_d: [S, D] f32 output AP.  x_d: [S, D] f32 input AP.
    w_d: dict q/k/v -> [D, D] f32 weight AP (torch Linear layout: out = x @ W^T).
    ln_d: dict qw/qb/kw/kb -> [D] f32 LN param APs.
    """
    nc = tc.nc

    with (
        tc.tile_pool(name="const", bufs=1) as const,
        tc.tile_pool(name="big", bufs=1) as big,
        tc.tile_pool(name="wtmp", bufs=3) as wtmp,
        tc.tile_pool(name="stat", bufs=6) as stat,
        tc.tile_pool(name="attn", bufs=2) as attn,
        tc.tile_pool(name="small", bufs=4) as small,
    ):
        # --- scalar-ring loads first: LN params + weights (land before any
        # transpose needs the DMA device)
        lnp = {}
        for key, nm in (("qw", "qnw"), ("qb", "qnb"), ("kw", "knw"),
                        ("kb", "knb")):
            t = const.tile([P, 1], F32, tag=nm, name=nm)
            nc.scalar.dma_start(t, ln_d[key][:, None])
            lnp[key] = t
        w32 = {}
        for name in ("k", "q", "v"):  # k first: head of the critical path
            w32[name] = wtmp.tile([P, P], F32, tag=f"w32_{name}",
                                  name=f"w32_{name}")
            nc.scalar.dma_start(w32[name], w_d[name])

        # --- x cast-loads on the GPSIMD SWDGE ring, 4 quarter chunks
        x_r = x_d.rearrange("(t p) d -> p t d", p=P)
        x16q = []
        for c in range(NQ):
            x16c = big.tile([P, TPQ, P], F16, tag=f"x16_{c}", name=f"x16_{c}")
            nc.gpsimd.dma_start(x16c, x_r[:, c * TPQ:(c + 1) * TPQ, :])
            x16q.append(x16c)

        # --- sync ring, need-order: wt_k, x quarters, wt_q, wt_v
        WT = {}

        def w_cast_transpose(name):
            w16 = wtmp.tile([P, P], F16, tag=f"w16_{name}", name=f"w16_{name}")
            nc.vector.tensor_copy(w16, w32[name])
            wt = const.tile([P, P], F16, tag=f"wt_{name}", name=f"wt_{name}")
            nc.sync.dma_start_transpose(wt, w16)
            WT[name] = wt

        w_cast_transpose("k")
        xTq = []
        for c in range(NQ):
            xtc = big.tile([P, TPQ, P], F16, tag=f"xT_{c}", name=f"xT_{c}")
            nc.sync.dma_start_transpose(
                xtc, x16q[c].rearrange("p t d -> p (t d)"))
            xTq.append(xtc)
        w_cast_transpose("q")
        w_cast_transpose("v")

        # --- projections + layernorm, k-first
        rawq = big.tile([P, NT, P], F16, tag="rawq")
        rawk = big.tile([P, NT, P], F16, tag="rawk")
        v16 = big.tile([P, NT, P + 1], F16, tag="v16")  # [:, t, P] = 1.0
        nc.vector.memset(v16[:, :, P:P + 1], 1.0)
        mv = {n: big.tile([P, NT, 2], F32, tag=f"mv_{n}", name=f"mv_{n}")
              for n in ("q", "k")}
        rstd = {n: big.tile([P, NT], F32, tag=f"rstd_{n}", name=f"rstd_{n}")
                for n in ("q", "k")}
        nmr = {n: big.tile([P, NT], F32, tag=f"nmr_{n}", name=f"nmr_{n}")
               for n in ("q", "k")}
        s1 = {n: big.tile([P, NT, P], F16, tag=f"s1_{n}", name=f"s1_{n}")
              for n in ("q", "k")}
        raw = {"q": rawq, "k": rawk}

        pps_ref = [None]

        def project(name, t):
            ps = pps_ref[0].tile([P, P], F32, tag=f"p_{name}",
                                 name=f"ps_{name}")
            nc.tensor.matmul(ps, lhsT=xTq[t // TPQ][:, t % TPQ, :],
                             rhs=WT[name], start=True, stop=True)
            if name == "v":
                # split v evacuations ACT/DVE: DVE also carries the bn stats
                # (317ns/tile), ACT the staging (292); either alone would
                # pace the whole in-order PE projection stream
                if t % 2 == 0:
                    nc.vector.tensor_copy(v16[:, t, :P], ps)
                else:
                    nc.scalar.activation(v16[:, t, :P], ps, AF.Copy)
                return
            nc.scalar.activation(raw[name][:, t, :], ps, AF.Copy)
            st = stat.tile([P, 6], F32, tag="st")
            nc.vector.bn_stats(st, raw[name][:, t, :])
            nc.vector.bn_aggr(mv[name][:, t, :], st)

        def rsqrt_batch(name, lo, hi):
            # rsqrt(v) = exp(-0.5 * ln(v)), batched over tiles [lo, hi)
            vare = stat.tile([P, NT], F32, tag=f"vare_{name}{lo}",
                             name=f"vare_{name}{lo}")
            nc.vector.tensor_scalar_add(vare[:, lo:hi],
                                        mv[name][:, lo:hi, 1], EPS)
            nc.scalar.activation(rstd[name][:, lo:hi], vare[:, lo:hi], AF.Ln)
            nc.scalar.activation(rstd[name][:, lo:hi], rstd[name][:, lo:hi],
                                 AF.Exp, scale=-0.5)
            nc.vector.scalar_tensor_tensor(
                nmr[name][:, lo:hi], in0=mv[name][:, lo:hi, 0], scalar=-1.0,
                in1=rstd[name][:, lo:hi], op0=ALU.mult, op1=ALU.mult)

        def ln_apply(name, ts):
            for t in ts:
                nc.vector.tensor_scalar(
                    s1[name][:, t, :], in0=raw[name][:, t, :],
                    scalar1=rstd[name][:, t:t + 1],
                    scalar2=nmr[name][:, t:t + 1],
                    op0=ALU.mult, op1=ALU.add)

        def transpose_fold(name, src_lo_tile, n_tiles, wsb, bsb, dst_tag):
            # xbar-transpose [s, h] -> [h, s] then fold LN weight/bias on
            # GPSIMD (two per-partition scalars). Pool is idle all prologue;
            # a DVE fold would stall the in-order DVE queue ~2.5us behind
            # each transpose's DGE+DMA+sem latency, delaying later stats.
            s1f = s1[name].rearrange("p t h -> p (t h)")
            pre = big.tile([P, n_tiles, P], F16, tag=f"{dst_tag}_pre",
                           name=f"{dst_tag}_pre")
            nc.sync.dma_start_transpose(
                pre, s1f[:, src_lo_tile * P:(src_lo_tile + n_tiles) * P])
            dst = big.tile([P, n_tiles, P], F16, tag=dst_tag, name=dst_tag)
            nc.gpsimd.tensor_scalar(
                dst.rearrange("h t s -> h (t s)"),
                in0=pre.rearrange("h t s -> h (t s)"),
                scalar1=wsb, scalar2=bsb, op0=ALU.mult, op1=ALU.add)
            return dst

        kTq = [None, None, None, None]
        qTq = [None, None, None, None]

        def finish_q_quarter(c):
            # stats for tiles of quarter c are already in; rsqrt + apply +
            # transpose + fold
            rsqrt_batch("q", c * TPQ, (c + 1) * TPQ)
            ln_apply("q", range(c * TPQ, (c + 1) * TPQ))
            qTq[c] = transpose_fold("q", c * TPQ, TPQ, lnp["qw"], lnp["qb"],
                                    f"qT{c}")

        # Three scoped single-tag PSUM pools, 8 banks deep each: shallower
        # pools stall the projection pipeline on PSUM-slot recycling (the
        # proj->evac->free sem chain is ~720ns per slot turn; 8 slots keep
        # the evac engine the pacer). Phase order k -> q -> v: attention
        # needs all of kT first, qT quarter 0 next, and v only from its
        # second PV slot onward.
        with tc.tile_pool(name="ppsk", bufs=8, space="PSUM") as ppsk:
            pps_ref[0] = ppsk
            for t in range(NT):
                project("k", t)
        rsqrt_batch("k", 0, NT)
        ln_apply("k", range(NT))
        # quarter-granular: each Pool fold is ~1.5us, so the serial Pool
        # fold chain (k quarters then q quarters) finishes before the v
        # phase drains
        for qq in range(NQ):
            kTq[qq] = transpose_fold("k", qq * TPQ, TPQ, lnp["kw"],
                                     lnp["kb"], f"kT{qq}")
        with tc.tile_pool(name="ppsq", bufs=8, space="PSUM") as ppsq:
            pps_ref[0] = ppsq
            # q quarter 0 chain first (attention group 0 needs it)
            for t in range(TPQ):
                project("q", t)
            finish_q_quarter(0)
            for t in range(TPQ, NT):
                project("q", t)
            for c in range(1, NQ):
                finish_q_quarter(c)
        with tc.tile_pool(name="ppsv", bufs=8, space="PSUM") as ppsv:
            pps_ref[0] = ppsv
            for t in range(NT):
                project("v", t)

        # --- attention
        with (
            tc.tile_pool(name="qkps", bufs=2, space="PSUM") as qkps,
            tc.tile_pool(name="pvps", bufs=4, space="PSUM") as pvps,
        ):
            expT = [None, None]   # per-group expT tiles (bufs=2 pool)
            pvt = {}              # (g % 2, ii) -> live PV psum tile
            osb_g = [None]        # current group's batched output staging

            def emit_qk_exp(g, jp):
                if jp == 0:
                    expT[g % 2] = attn.tile([P, NT, GW], F16, tag="expt",
                                            name="expT")
                ps = qkps.tile([P, 2, GW], F32, tag="qk", name="qk_ps")
                qg = qTq[g // 2]
                qoff = (g % 2) * TPG
                for h in range(2):
                    jt = 2 * jp + h
                    nc.tensor.matmul(
                        ps[:, h, :], lhsT=kTq[jt // TPQ][:, jt % TPQ, :],
                        rhs=qg[:, qoff:qoff + TPG, :].rearrange(
                            "h t s -> h (t s)"),
                        start=True, stop=True)
                nc.scalar.activation(
                    expT[g % 2][:, 2 * jp:2 * jp + 2, :], ps, AF.Exp,
                    scale=ISQRT_D)

            def emit_pv(g, jp):
                # 8 PV accumulation steps for stream slot (g, jp): chunks
                # c = 2*jp, 2*jp+1 into all four of group g's accumulators.
                # Column P of each accumulator is the softmax row-sum (ones
                # column in v16).
                e = expT[g % 2]
                for ii in range(TPG):
                    if jp == 0:
                        pvt[(g % 2, ii)] = pvps.tile(
                            [P, P + 1], F32, tag="pv", name="pv_acc")
                    ops = pvt[(g % 2, ii)]
                    for c in (2 * jp, 2 * jp + 1):
                        nc.tensor.matmul(
                            ops, lhsT=e[:, c, ii * P:(ii + 1) * P],
                            rhs=v16[:, c, :],
                            start=(c == 0), stop=(c == NT - 1))
                    if jp == JP - 1:
                        if ii == 0:
                            osb_g[0] = small.tile([P, TPG, P], F32, tag="osb",
                                                  name="osb")
                        rsum = small.tile([P, 1], F32, tag="rsum")
                        nc.vector.reciprocal(rsum, ops[:, P:P + 1])
                        nc.vector.tensor_scalar_mul(osb_g[0][:, ii, :],
                                                    ops[:, :P], rsum)
                        if ii == TPG - 1:
                            # one batched out DMA per group on the sync ring
                            nc.sync.dma_start(
                                out_d[g * GW:(g + 1) * GW, :].rearrange(
                                    "(t p) d -> p t d", p=P),
                                osb_g[0])

            NS = NG * JP
            for n in range(NS + 2):
                if n < NS:
                    emit_qk_exp(n // JP, n % JP)
                if n >= 2:
                    m = n - 2
                    emit_pv(m // JP, m % JP)


_NC_CACHE = None


def _build():
    global _NC_CACHE
    if _NC_CACHE is not None:
        return _NC_CACHE
    nc = bacc.Bacc("TRN2", target_bir_lowering=False, debug=False)
    x = nc.dram_tensor("x", [S, D], F32, kind="ExternalInput").ap()
    wq = nc.dram_tensor("Wq", [D, D], F32, kind="ExternalInput").ap()
    wk = nc.dram_tensor("Wk", [D, D], F32, kind="ExternalInput").ap()
    wv = nc.dram_tensor("Wv", [D, D], F32, kind="ExternalInput").ap()
    qn_w = nc.dram_tensor("qn_w", [D], F32, kind="ExternalInput").ap()
    qn_b = nc.dram_tensor("qn_b", [D], F32, kind="ExternalInput").ap()
    kn_w = nc.dram_tensor("kn_w", [D], F32, kind="ExternalInput").ap()
    kn_b = nc.dram_tensor("kn_b", [D], F32, kind="ExternalInput").ap()
    out = nc.dram_tensor("out", [S, D], F32, kind="ExternalOutput").ap()
    with tile.TileContext(nc) as tc:
        _build_attention(
            tc, out, x,
            {"q": wq, "k": wk, "v": wv},
            {"qw": qn_w, "qb": qn_b, "kw": kn_w, "kb": kn_b},
        )
    nc.compile()
    _NC_CACHE = nc
    return nc


def kernel(x, Wq, Wk, Wv, qn_w, qn_b, kn_w, kn_b, _run_kwargs=None):
    nc = _build()
    x = np.asarray(x, dtype=np.float32)
    shared = {
        "Wq": np.ascontiguousarray(np.asarray(Wq, np.float32)),
        "Wk": np.ascontiguousarray(np.asarray(Wk, np.float32)),
        "Wv": np.ascontiguousarray(np.asarray(Wv, np.float32)),
        "qn_w": np.ascontiguousarray(np.asarray(qn_w, np.float32)),
        "qn_b": np.ascontiguousarray(np.asarray(qn_b, np.float32)),
        "kn_w": np.ascontiguousarray(np.asarray(kn_w, np.float32)),
        "kn_b": np.ascontiguousarray(np.asarray(kn_b, np.float32)),
    }
    in_maps = [
        {"x": np.ascontiguousarray(x[b]), **shared} for b in range(B)
    ]
    res = run_bass_kernel_spmd(nc, in_maps, core_ids=list(range(N_CORES)),
                               **(_run_kwargs or {}))
    out = np.stack([res.results[b]["out"] for b in range(B)], axis=0)
    if _run_kwargs:
        kernel.last_results = res
    return out.astype(np.float32)


# revision 3
# speedup vs baseline: 1.4542x; 1.4542x over previous
"""Trainium2 Bass kernel for nn_AttentionHead (B=8, S=4096, D=128).

Sharding: data-parallel over the batch dim — 1 batch element per NeuronCore,
8 cores, SPMD (same NEFF, different x slice), weights replicated. No
collectives.

The kernel is ACT-bound: softmax needs S^2 = 16.7M exps per core ~= 109us of
Activation-engine time (1.2GHz x 128 lanes). Everything else is scheduled to
hide under the exp stream and to minimize the prologue before the first exp.
Cost-model time ~190us/core vs 345us for the v1 (exp-transpose) design.

Attention (transposed scores — the key structural change vs v1):
    scoresT[jt, q_group] = kT_tile^T @ qT_group   (groups of 512 queries)
so ACT's exp writes f16 straight into the [j, q] layout PV needs for lhsT.
v1 instead DMA-xbar-transposed the exp matrix (32MB/core through the sync
ring), which made the DMA engines a co-bottleneck and serialized exp->PV.
Flat stream over n = g*16 + jp: QK pair (N=512) -> one exp [128, 1024]
(PSUM->SBUF, scale=1/sqrt(D), no max-subtraction: LN'd q/k keep scores in
f16 exp range) -> 8 PV matmuls for stream slot n-2 (lag 2 exp ops). All
four PV accumulators of a group are live at once (pvps bufs=4 x 1 bank;
qkps bufs=2 x 2 banks = all 8 banks), so PV chases exp inside its own
group and the post-stream tail is one PV slot. This exp shape is PSUM-
optimal: PV fundamentally needs 4 live banks (each i-tile's accumulation
spans its whole group) and QK/exp need double-buffering, so free-1024 exps
(128 ops x 1038ns = 133us, zero inter-exp stall in the cost model) are the
best reachable. PV's rhs is v16 with an appended ones column: column P of
each accumulator accumulates the softmax denominator for free. Normalize
by 1/rowsum on DVE; one batched output DMA per group on the sync ring
(SWDGE pays ~1us descriptor-gen per DMA on the GPSIMD engine, which would
serialize into the kernel tail; the scalar ring's DMAs issue from the ACT
sequencer and head-of-line block the exp stream).

Prologue (~51us to first exp). Hard-won scheduling facts (cost-model
traces; the tile scheduler is invariant to emission order of independent
ops, so only structure matters):
  - An xbar DmaTransposeAnt waits for ALL prior in-flight DMA completions
    (+~0.9us sem prop): the sync ring carries ONLY the x transposes plus
    the post-LN kT/qT transposes; LN params + weights go on the scalar
    ring (tiny, land first); x cast-loads (f32->f16 SWDGE) on the GPSIMD
    ring in 4 quarter tiles so the k projections chase the chunks.
  - Weight transposes run on the PE (identity matmul, PSUM bounce) — an
    xbar wt transpose ahead of the x chain costs ~3.5us of ring pipeline.
  - Phases use scoped single/dual-tag PSUM pools 8 banks deep (shallower
    pools stall the proj->evac->free slot-recycle sem chain, ~740ns/turn):
    k phase (8 banks), q phase (8), v phase (8), in that order — attention
    needs all of kT first, qT per-group-pair, v only from PV slot 2 on.
  - Per tensor: project (PE) -> stage raw f16 (ACT; also frees PSUM) ->
    bn stats from the staged f16 (DVE) -> ONE batched
    rsqrt = exp(-0.5*ln(var+eps)) per tensor (Ln/Exp share the ACT table
    set with the attention Exp -> ~1 table load total; finer-grained
    per-quarter rsqrt chains measure strictly worse) -> LN apply via DVE
    tensor_scalar (two per-partition scalars) -> xbar-transpose quarters
    -> LN weight/bias fold on GPSIMD (idle in the prologue; DVE folds
    stall its in-order queue behind each transpose's DGE+sem latency).
  - v evacuations split ACT/DVE by parity: either engine alone paces the
    in-order PE projection stream.

All SBUF pools stay open for the whole kernel (SBUF-slot reuse attaches
release waits to DMAs loading into recycled space; walrus rejects DMAs
with too many sync waits). Only PSUM pools are scoped. All xbar transposes
go on the sync HWDGE ring (concurrent transposes on the sync+scalar rings
corrupt data on HW, per v1 bisection).
"""

import math

import numpy as np

from concourse import bacc
import concourse.mybir as mybir
import concourse.tile as tile
from concourse.bass_utils import run_bass_kernel_spmd

F16 = mybir.dt.float16
F32 = mybir.dt.float32
AF = mybir.ActivationFunctionType
ALU = mybir.AluOpType

B, S, D = 8, 4096, 128
P = 128
NT = S // P    # 32 s-tiles
NQ = 4         # x/q quarter chunks
TPQ = NT // NQ  # 8 tiles per quarter
EPS = 1e-5
ISQRT_D = 1.0 / math.sqrt(D)
N_CORES = 8
GW = 512           # queries per attention group
NG = S // GW       # 8 groups
TPG = GW // P      # 4 i-tiles per group
JP = NT // 2       # 16 jt-pairs (= exp ops = stream slots) per group


def _build_attention(tc, out_d, x_d, w_d, ln_d):
    """Emit the single-core attention program.

    out_d: [S, D] f32 output AP.  x_d: [S, D] f32 input AP.
    w_d: dict q/k/v -> [D, D] f32 weight AP (torch Linear layout: out = x @ W^T).
    ln_d: dict qw/qb/kw/kb -> [D] f32 LN param APs.
    """
    nc = tc.nc

    with (
        tc.tile_pool(name="const", bufs=1) as const,
        tc.tile_pool(name="big", bufs=1) as big,
        tc.tile_pool(name="wtmp", bufs=3) as wtmp,
        tc.tile_pool(name="stat", bufs=6) as stat,
        tc.tile_pool(name="attn", bufs=2) as attn,
        tc.tile_pool(name="small", bufs=4) as small,
    ):
        # --- scalar-ring loads first: LN params + weights (land before any
        # transpose needs the DMA device)
        lnp = {}
        for key, nm in (("qw", "qnw"), ("qb", "qnb"), ("kw", "knw"),
                        ("kb", "knb")):
            t = const.tile([P, 1], F32, tag=nm, name=nm)
            nc.scalar.dma_start(t, ln_d[key][:, None])
            lnp[key] = t
        w32 = {}
        for name in ("k", "q", "v"):  # k first: head of the critical path
            w32[name] = wtmp.tile([P, P], F32, tag=f"w32_{name}",
                                  name=f"w32_{name}")
            nc.scalar.dma_start(w32[name], w_d[name])

        # --- x cast-loads on the GPSIMD SWDGE ring, 4 quarter chunks
        x_r = x_d.rearrange("(t p) d -> p t d", p=P)
        x16q = []
        for c in range(NQ):
            x16c = big.tile([P, TPQ, P], F16, tag=f"x16_{c}", name=f"x16_{c}")
            nc.gpsimd.dma_start(x16c, x_r[:, c * TPQ:(c + 1) * TPQ, :])
            x16q.append(x16c)

        # identity for PE transposes (ones + affine_select p==j on GPSIMD)
        ident = const.tile([P, P], F16, tag="ident", name="ident")
        nc.gpsimd.memset(ident, 1.0)
        nc.gpsimd.affine_select(ident, ident, pattern=[[-1, P]],
                                compare_op=ALU.is_equal, fill=0.0,
                                base=0, channel_multiplier=1)

        # --- weight transposes on the PE (identity matmul): keeps the sync
        # ring x-transposes-only. A wt xbar transpose ahead of the x chain
        # costs ~3.5us of ring pipeline + sem churn before the first x
        # transpose can fire.
        WT = {}
        with tc.tile_pool(name="wps", bufs=1, space="PSUM") as wps:
            for name in ("k", "q", "v"):
                w16 = wtmp.tile([P, P], F16, tag=f"w16_{name}",
                                name=f"w16_{name}")
                nc.vector.tensor_copy(w16, w32[name])
                wt_ps = wps.tile([P, P], F16, tag=f"wtp_{name}",
                                 name=f"wtp_{name}")
                nc.tensor.transpose(wt_ps, w16, ident)
                wt = const.tile([P, P], F16, tag=f"wt_{name}",
                                name=f"wt_{name}")
                nc.vector.tensor_copy(wt, wt_ps)
                WT[name] = wt

        xTq = []
        for c in range(NQ):
            xtc = big.tile([P, TPQ, P], F16, tag=f"xT_{c}", name=f"xT_{c}")
            nc.sync.dma_start_transpose(
                xtc, x16q[c].rearrange("p t d -> p (t d)"))
            xTq.append(xtc)

        # --- projections + layernorm, k-first
        rawq = big.tile([P, NT, P], F16, tag="rawq")
        rawk = big.tile([P, NT, P], F16, tag="rawk")
        v16 = big.tile([P, NT, P + 1], F16, tag="v16")  # [:, t, P] = 1.0
        nc.vector.memset(v16[:, :, P:P + 1], 1.0)
        mv = {n: big.tile([P, NT, 2], F32, tag=f"mv_{n}", name=f"mv_{n}")
              for n in ("q", "k")}
        rstd = {n: big.tile([P, NT], F32, tag=f"rstd_{n}", name=f"rstd_{n}")
                for n in ("q", "k")}
        nmr = {n: big.tile([P, NT], F32, tag=f"nmr_{n}", name=f"nmr_{n}")
               for n in ("q", "k")}
        s1 = {n: big.tile([P, NT, P], F16, tag=f"s1_{n}", name=f"s1_{n}")
              for n in ("q", "k")}
        raw = {"q": rawq, "k": rawk}

        pps_ref = [None]

        def project(name, t):
            ps = pps_ref[0].tile([P, P], F32, tag=f"p_{name}",
                                 name=f"ps_{name}")
            nc.tensor.matmul(ps, lhsT=xTq[t // TPQ][:, t % TPQ, :],
                             rhs=WT[name], start=True, stop=True)
            if name == "v":
                # split v evacuations ACT/DVE: DVE also carries the bn stats
                # (317ns/tile), ACT the staging (292); either alone would
                # pace the whole in-order PE projection stream
                if t % 2 == 0:
                    nc.vector.tensor_copy(v16[:, t, :P], ps)
                else:
                    nc.scalar.activation(v16[:, t, :P], ps, AF.Copy)
                return
            nc.scalar.activation(raw[name][:, t, :], ps, AF.Copy)
            st = stat.tile([P, 6], F32, tag="st")
            nc.vector.bn_stats(st, raw[name][:, t, :])
            nc.vector.bn_aggr(mv[name][:, t, :], st)

        def rsqrt_batch(name, lo, hi):
            # rsqrt(v) = exp(-0.5 * ln(v)), batched over tiles [lo, hi)
            vare = stat.tile([P, NT], F32, tag=f"vare_{name}{lo}",
                             name=f"vare_{name}{lo}")
            nc.vector.tensor_scalar_add(vare[:, lo:hi],
                                        mv[name][:, lo:hi, 1], EPS)
            nc.scalar.activation(rstd[name][:, lo:hi], vare[:, lo:hi], AF.Ln)
            nc.scalar.activation(rstd[name][:, lo:hi], rstd[name][:, lo:hi],
                                 AF.Exp, scale=-0.5)
            nc.vector.scalar_tensor_tensor(
                nmr[name][:, lo:hi], in0=mv[name][:, lo:hi, 0], scalar=-1.0,
                in1=rstd[name][:, lo:hi], op0=ALU.mult, op1=ALU.mult)

        def ln_apply(name, ts):
            for t in ts:
                nc.vector.tensor_scalar(
                    s1[name][:, t, :], in0=raw[name][:, t, :],
                    scalar1=rstd[name][:, t:t + 1],
                    scalar2=nmr[name][:, t:t + 1],
                    op0=ALU.mult, op1=ALU.add)

        def transpose_fold(name, src_lo_tile, n_tiles, wsb, bsb, dst_tag):
            # xbar-transpose [s, h] -> [h, s] then fold LN weight/bias on
            # GPSIMD (two per-partition scalars). Pool is idle all prologue;
            # a DVE fold would stall the in-order DVE queue ~2.5us behind
            # each transpose's DGE+DMA+sem latency, delaying later stats.
            s1f = s1[name].rearrange("p t h -> p (t h)")
            pre = big.tile([P, n_tiles, P], F16, tag=f"{dst_tag}_pre",
                           name=f"{dst_tag}_pre")
            nc.sync.dma_start_transpose(
                pre, s1f[:, src_lo_tile * P:(src_lo_tile + n_tiles) * P])
            dst = big.tile([P, n_tiles, P], F16, tag=dst_tag, name=dst_tag)
            nc.gpsimd.tensor_scalar(
                dst.rearrange("h t s -> h (t s)"),
                in0=pre.rearrange("h t s -> h (t s)"),
                scalar1=wsb, scalar2=bsb, op0=ALU.mult, op1=ALU.add)
            return dst

        kTq = [None, None, None, None]
        qTq = [None, None, None, None]

        def finish_q_quarter(c):
            # stats for tiles of quarter c are already in; rsqrt + apply +
            # transpose + fold
            rsqrt_batch("q", c * TPQ, (c + 1) * TPQ)
            ln_apply("q", range(c * TPQ, (c + 1) * TPQ))
            qTq[c] = transpose_fold("q", c * TPQ, TPQ, lnp["qw"], lnp["qb"],
                                    f"qT{c}")

        # Three scoped single-tag PSUM pools, 8 banks deep each: shallower
        # pools stall the projection pipeline on PSUM-slot recycling (the
        # proj->evac->free sem chain is ~720ns per slot turn; 8 slots keep
        # the evac engine the pacer). Phase order k -> q -> v: attention
        # needs all of kT first, qT quarter 0 next, and v only from its
        # second PV slot onward.
        with tc.tile_pool(name="ppsk", bufs=8, space="PSUM") as ppsk:
            pps_ref[0] = ppsk
            for t in range(NT):
                project("k", t)
        rsqrt_batch("k", 0, NT)
        ln_apply("k", range(NT))
        # quarter-granular: each Pool fold is ~1.5us, so the serial Pool
        # fold chain (k quarters then q quarters) finishes before the v
        # phase drains
        for qq in range(NQ):
            kTq[qq] = transpose_fold("k", qq * TPQ, TPQ, lnp["kw"],
                                     lnp["kb"], f"kT{qq}")
        with tc.tile_pool(name="ppsq", bufs=8, space="PSUM") as ppsq:
            pps_ref[0] = ppsq
            for t in range(NT):
                project("q", t)
            # one batched rsqrt + apply for all 32 q tiles (2 ACT ops
            # instead of 8), then the four transpose+fold chains
            rsqrt_batch("q", 0, NT)
            ln_apply("q", range(NT))
            for c in range(NQ):
                qTq[c] = transpose_fold("q", c * TPQ, TPQ, lnp["qw"],
                                        lnp["qb"], f"qT{c}")
        with tc.tile_pool(name="ppsv", bufs=8, space="PSUM") as ppsv:
            pps_ref[0] = ppsv
            for t in range(NT):
                project("v", t)

        # --- attention
        with (
            tc.tile_pool(name="qkps", bufs=2, space="PSUM") as qkps,
            tc.tile_pool(name="pvps", bufs=4, space="PSUM") as pvps,
        ):
            expT = [None, None]   # per-group expT tiles (bufs=2 pool)
            pvt = {}              # (g % 2, ii) -> live PV psum tile
            osb_g = [None]        # current group's batched output staging

            def emit_qk_exp(g, jp):
                if jp == 0:
                    expT[g % 2] = attn.tile([P, NT, GW], F16, tag="expt",
                                            name="expT")
                ps = qkps.tile([P, 2, GW], F32, tag="qk", name="qk_ps")
                qg = qTq[g // 2]
                qoff = (g % 2) * TPG
                for h in range(2):
                    jt = 2 * jp + h
                    nc.tensor.matmul(
                        ps[:, h, :], lhsT=kTq[jt // TPQ][:, jt % TPQ, :],
                        rhs=qg[:, qoff:qoff + TPG, :].rearrange(
                            "h t s -> h (t s)"),
                        start=True, stop=True)
                nc.scalar.activation(
                    expT[g % 2][:, 2 * jp:2 * jp + 2, :], ps, AF.Exp,
                    scale=ISQRT_D)

            def emit_pv(g, jp):
                # 8 PV accumulation steps for stream slot (g, jp): chunks
                # c = 2*jp, 2*jp+1 into all four of group g's accumulators.
                # Column P of each accumulator is the softmax row-sum (ones
                # column in v16).
                e = expT[g % 2]
                for ii in range(TPG):
                    if jp == 0:
                        pvt[(g % 2, ii)] = pvps.tile(
                            [P, P + 1], F32, tag="pv", name="pv_acc")
                    ops = pvt[(g % 2, ii)]
                    for c in (2 * jp, 2 * jp + 1):
                        nc.tensor.matmul(
                            ops, lhsT=e[:, c, ii * P:(ii + 1) * P],
                            rhs=v16[:, c, :],
                            start=(c == 0), stop=(c == NT - 1))
                    if jp == JP - 1:
                        if ii == 0:
                            osb_g[0] = small.tile([P, TPG, P], F32, tag="osb",
                                                  name="osb")
                        rsum = small.tile([P, 1], F32, tag="rsum")
                        nc.vector.reciprocal(rsum, ops[:, P:P + 1])
                        nc.vector.tensor_scalar_mul(osb_g[0][:, ii, :],
                                                    ops[:, :P], rsum)
                        if ii == TPG - 1:
                            # one batched out DMA per group on the sync ring
                            nc.sync.dma_start(
                                out_d[g * GW:(g + 1) * GW, :].rearrange(
                                    "(t p) d -> p t d", p=P),
                                osb_g[0])

            NS = NG * JP
            for n in range(NS + 2):
                if n < NS:
                    emit_qk_exp(n // JP, n % JP)
                if n >= 2:
                    m = n - 2
                    emit_pv(m // JP, m % JP)


_NC_CACHE = None


def _build():
    global _NC_CACHE
    if _NC_CACHE is not None:
        return _NC_CACHE
    nc = bacc.Bacc("TRN2", target_bir_lowering=False, debug=False)
    x = nc.dram_tensor("x", [S, D], F32, kind="ExternalInput").ap()
    wq = nc.dram_tensor("Wq", [D, D], F32, kind="ExternalInput").ap()
    wk = nc.dram_tensor("Wk", [D, D], F32, kind="ExternalInput").ap()
    wv = nc.dram_tensor("Wv", [D, D], F32, kind="ExternalInput").ap()
    qn_w = nc.dram_tensor("qn_w", [D], F32, kind="ExternalInput").ap()
    qn_b = nc.dram_tensor("qn_b", [D], F32, kind="ExternalInput").ap()
    kn_w = nc.dram_tensor("kn_w", [D], F32, kind="ExternalInput").ap()
    kn_b = nc.dram_tensor("kn_b", [D], F32, kind="ExternalInput").ap()
    out = nc.dram_tensor("out", [S, D], F32, kind="ExternalOutput").ap()
    with tile.TileContext(nc) as tc:
        _build_attention(
            tc, out, x,
            {"q": wq, "k": wk, "v": wv},
            {"qw": qn_w, "qb": qn_b, "kw": kn_w, "kb": kn_b},
        )
    nc.compile()
    _NC_CACHE = nc
    return nc


def kernel(x, Wq, Wk, Wv, qn_w, qn_b, kn_w, kn_b, _run_kwargs=None):
    nc = _build()
    x = np.asarray(x, dtype=np.float32)
    shared = {
        "Wq": np.ascontiguousarray(np.asarray(Wq, np.float32)),
        "Wk": np.ascontiguousarray(np.asarray(Wk, np.float32)),
        "Wv": np.ascontiguousarray(np.asarray(Wv, np.float32)),
        "qn_w": np.ascontiguousarray(np.asarray(qn_w, np.float32)),
        "qn_b": np.ascontiguousarray(np.asarray(qn_b, np.float32)),
        "kn_w": np.ascontiguousarray(np.asarray(kn_w, np.float32)),
        "kn_b": np.ascontiguousarray(np.asarray(kn_b, np.float32)),
    }
    in_maps = [
        {"x": np.ascontiguousarray(x[b]), **shared} for b in range(B)
    ]
    res = run_bass_kernel_spmd(nc, in_maps, core_ids=list(range(N_CORES)),
                               **(_run_kwargs or {}))
    out = np.stack([res.results[b]["out"] for b in range(B)], axis=0)
    if _run_kwargs:
        kernel.last_results = res
    return out.astype(np.float32)
